# revision 1
# baseline (speedup 1.0000x reference)
"""Trainium2 Bass kernel for windowed multi-agent attention (Swin-style).

Full-input contract: kernel(**inputs) takes the unsharded inputs and returns
the unsharded output. Internally shards over the H axis across 8 NeuronCores
(fully data-parallel over window rows), builds one SPMD Bass program, and
runs it via run_bass_kernel_spmd.

Math (per window of 4x4 spatial, 4 agents => T=64 tokens; the 5th padded
agent is masked out everywhere in the reference, so it is simply dropped):
  xw (64, 256) -> qkv -> 4 heads of d=64 -> softmax(q k^T * scale + bias) v
  -> out proj (256, 256) -> back to NCHW.
"""

import numpy as np

HEADS = 4
WIN = 4
MAX_N = 5
DIM = 256
N_AGENTS = 4
H = W = 128
N_CORES = 8
T = N_AGENTS * WIN * WIN          # 64 valid tokens per window
HS = 16                           # H rows per core
N_STRIPS = 4                      # window-rows per core (4 H-rows each)
N_GROUPS = 4                      # groups of 8 windows per strip
GW = 8                            # windows per group
NT = GW * T                       # tokens per group = 512


def _rel_pos_index(N, wh, ww, md, mh, mw):
    cd, ch, cw = np.arange(N), np.arange(wh), np.arange(ww)
    coords = np.stack(np.meshgrid(cd, ch, cw, indexing="ij")).reshape(3, -1)
    rel = (coords[:, :, None] - coords[:, None, :]).transpose(1, 2, 0).astype(np.int64)
    rel[..., 0] += md - 1
    rel[..., 1] += mh - 1
    rel[..., 2] += mw - 1
    rel[..., 0] *= (2 * mh - 1) * (2 * mw - 1)
    rel[..., 1] *= 2 * mw - 1
    return rel.sum(-1)


def _build_bias_stacks(bias_table):
    """bias (heads, 64, 64) for the valid 4 agents; returns two (128, 512)
    stacks: stack A = heads (0, 1), B = (2, 3); rows = head-pair x t_q,
    cols = 8 windows tiled x t_k (bias identical for every window)."""
    rpi = _rel_pos_index(MAX_N, WIN, WIN, MAX_N, WIN, WIN)  # (80, 80)
    b = bias_table[rpi]                                     # (80, 80, HEADS)
    b = b[:T, :T].transpose(2, 0, 1).astype(np.float32)     # (HEADS, 64, 64)
    stacks = []
    for s in range(2):
        st = np.concatenate([b[2 * s], b[2 * s + 1]], axis=0)   # (128, 64)
        st = np.tile(st, (1, GW))                               # (128, 512)
        stacks.append(np.ascontiguousarray(np.exp(st)))
    return stacks


def _patch_tile_drain():
    """Walrus in this container rejects >1 sync-wait on the TileContext tail
    drain; split the waits across individual SP nops instead."""
    from concourse import tile as tile_mod
    from concourse.vector_clock import ScopedClock, VectorClock
    if getattr(tile_mod.TileContext, "_drain_patched", False):
        return

    def _patched(self, tick_clock, wait_clock):
        gc_ = tick_clock.global_clock
        n = len(gc_)
        for proc in range(n):
            tick = gc_[proc]
            if tick <= 0:
                continue
            vc = VectorClock([0] * n)
            vc.require_at_least(proc, tick)
            nop_inst = self.nc.sync.nop(nofuse=True)
            wait_clock.add_sem_waits(nop_inst.ins, ScopedClock({None: vc}))
        self.nc.sync.drain()
        self.nc.all_engine_barrier()
        popped = self.nc._tile_sem_poison_stack.pop()
        assert popped is self._sem_poison
        self.nc.clear_and_free_semaphores(list(self.sems.allocated().values()))
        self.nc.all_engine_barrier()

    tile_mod.TileContext._drain_and_barrier = _patched
    tile_mod.TileContext._drain_patched = True


def _split_multi_waits(nc):
    """Walrus here allows only one sync-wait per instruction. Rewrite the BIR
    json: for each instruction with >1 on_wait, hoist the extras onto fresh
    single-wait Nops inserted just before it on the same engine."""
    import orjson
    orig = nc.to_json_bytes

    def patched():
        bj = orjson.loads(orig())
        counter = [0]
        for fn in bj.get("functions", []):
            for blk in fn.get("blocks", []):
                insts = blk.get("instructions", [])
                out = []
                for inst in insts:
                    si = inst.get("sync_info") or {}
                    waits = si.get("on_wait") or []
                    if len(waits) > 1:
                        for w in waits[:-1]:
                            counter[0] += 1
                            out.append({
                                "name": f"WSPL-{counter[0]}",
                                "opcode": "NoOp",
                                "engine": inst["engine"],
                                "ins": [],
                                "outs": [],
                                "sync_info": {"on_update": [], "on_wait": [w]},
                            })
                        si["on_wait"] = [waits[-1]]
                    out.append(inst)
                blk["instructions"] = out
        return orjson.dumps(bj)

    nc.to_json_bytes = patched
    return nc


def build_nc():
    import os
    from concourse import bass, mybir
    from concourse.tile import TileContext
    _patch_tile_drain()
    STAGE = os.environ.get("KSTAGE", "full")
    N_STRIPS_ = int(os.environ.get("KSTRIPS", N_STRIPS))
    N_GROUPS_ = int(os.environ.get("KGROUPS", N_GROUPS))
    _norm = os.environ.get("KNORM", "dve")
    _grpb = int(os.environ.get("KGRPB", "3"))
    _softb = int(os.environ.get("KSOFTB", "4"))
    _psb = int(os.environ.get("KPSB", "8"))
    _prio = int(os.environ.get("KPRIO", "0"))
    def stage_ge(s):
        order = ["qkv", "v", "sim", "soft", "tp", "av", "out", "full"]
        return order.index(STAGE) >= order.index(s)

    F32 = mybir.dt.float32
    BF16 = mybir.dt.bfloat16
    AX = mybir.AxisListType.X
    EXP = mybir.ActivationFunctionType.Exp

    nc = bass.Bass("TRN2", target_bir_lowering=False, debug=False,
                   num_devices=N_CORES)

    xs_d = nc.dram_tensor("xs", [N_AGENTS, DIM, HS, W], F32, kind="ExternalInput").ap()
    wq_d = nc.dram_tensor("wq", [DIM, DIM], F32, kind="ExternalInput").ap()
    wk_d = nc.dram_tensor("wk", [DIM, DIM], F32, kind="ExternalInput").ap()
    wv_d = nc.dram_tensor("wv", [DIM, DIM], F32, kind="ExternalInput").ap()
    wo_d = nc.dram_tensor("wo", [DIM, DIM], F32, kind="ExternalInput").ap()
    ba_d = nc.dram_tensor("biasA", [128, NT], F32, kind="ExternalInput").ap()
    bb_d = nc.dram_tensor("biasB", [128, NT], F32, kind="ExternalInput").ap()
    id_d = nc.dram_tensor("ident", [128, 128], F32, kind="ExternalInput").ap()
    out_d = nc.dram_tensor("out", [N_AGENTS, DIM, HS, W], F32, kind="ExternalOutput").ap()

    from contextlib import ExitStack
    with TileContext(nc) as tc, ExitStack() as _stk:
        cpool = _stk.enter_context(tc.tile_pool(name="consts", bufs=1))
        # weight tiles (bf16, cast during DMA via SWDGE)
        Wq = [[cpool.tile([128, 128], BF16, name=f"wq{c}{h}", tag=f"wq{c}{h}") for h in range(2)] for c in range(2)]
        Wk = [[cpool.tile([128, 128], BF16, name=f"wk{c}{h}", tag=f"wk{c}{h}") for h in range(2)] for c in range(2)]
        Wv = [cpool.tile([128, 256], BF16, name=f"wv{c}", tag=f"wv{c}") for c in range(2)]
        Wo = [[cpool.tile([128, 128], BF16, name=f"wo{c}{h}", tag=f"wo{c}{h}") for h in range(2)] for c in range(2)]
        biasA = cpool.tile([128, NT], BF16, name="biasA", tag="biasA")
        biasB = cpool.tile([128, NT], BF16, name="biasB", tag="biasB")
        ident = cpool.tile([128, 128], BF16, name="ident", tag="ident")
        for c in range(2):
            cs = slice(c * 128, (c + 1) * 128)
            for h in range(2):
                hs_ = slice(h * 128, (h + 1) * 128)
                nc.gpsimd.dma_start(out=Wq[c][h][:], in_=wq_d[cs, hs_])
                nc.gpsimd.dma_start(out=Wk[c][h][:], in_=wk_d[cs, hs_])
                nc.gpsimd.dma_start(out=Wo[c][h][:], in_=wo_d[cs, hs_])
            nc.gpsimd.dma_start(out=Wv[c][:], in_=wv_d[cs, :])
        nc.gpsimd.dma_start(out=biasA[:], in_=ba_d)
        nc.gpsimd.dma_start(out=biasB[:], in_=bb_d)
        nc.gpsimd.dma_start(out=ident[:], in_=id_d)

        strip = _stk.enter_context(tc.tile_pool(name="strip", bufs=2))
        grp = _stk.enter_context(tc.tile_pool(name="grp", bufs=_grpb))
        soft = _stk.enter_context(tc.tile_pool(name="soft", bufs=_softb))
        ps = _stk.enter_context(tc.tile_pool(name="ps", bufs=_psb, space="PSUM"))
        _norm_eng = nc.gpsimd if _norm == "pool" else nc.vector

        def load_strip(s, chunked=False):
            hs_sl = slice(s * WIN, (s + 1) * WIN)
            Traw = [strip.tile([128, 2048], F32, name=f"traw{c}", tag=f"traw{c}") for c in range(2)]
            Ttok = [strip.tile([128, 2048], BF16, name=f"ttok{c}", tag=f"ttok{c}") for c in range(2)]
            for c in range(2):
                rawd = Traw[c][:].rearrange("p (a i w) -> p a i w", a=4, i=4)
                if chunked:
                    # per-group chunks so group 0 compute starts after 1/4 of the DMA
                    for gch in range(N_GROUPS):
                        wsl = slice(gch * 32, (gch + 1) * 32)
                        for a_ in range(4):
                            src = xs_d[a_, c * 128:(c + 1) * 128, hs_sl, wsl]
                            nc.sync.dma_start(out=rawd[:, a_, :, wsl], in_=src)
                else:
                    src = xs_d[:, c * 128:(c + 1) * 128, hs_sl, :].transpose([1, 0, 2, 3])
                    nc.sync.dma_start(out=rawd, in_=src)
            # reorder (a, i, w128) -> (ww, a, i, j) token order; cast to bf16
            for c in range(2):
                tokv = Ttok[c][:].rearrange("p (w a i j) -> p a w i j", w=32, a=4, i=4, j=4)
                rawv = Traw[c][:].rearrange("p (a i w j) -> p a w i j", a=4, i=4, w=32, j=4)
                if chunked:
                    for gch in range(N_GROUPS):
                        wsl = slice(gch * 8, (gch + 1) * 8)
                        for a_ in range(4):
                            nc.gpsimd.tensor_copy(tokv[:, a_, wsl], rawv[:, a_, wsl])
                else:
                    for a_ in range(4):
                        nc.gpsimd.tensor_copy(tokv[:, a_], rawv[:, a_])
            return Ttok

        pending = load_strip(0, chunked=True)
        for s in range(N_STRIPS_):
            hs_sl = slice(s * WIN, (s + 1) * WIN)
            Ttok = pending
            if s + 1 < N_STRIPS_:
                pending = load_strip(s + 1, chunked=True)
            OS = [strip.tile([128, 2048], F32, name=f"os{c}", tag=f"os{c}") for c in range(2)]
            OR = [strip.tile([128, 2048], F32, name=f"or{c}", tag=f"or{c}") for c in range(2)]

            for g in range(N_GROUPS_):
                gt = slice(g * NT, (g + 1) * NT)
                tok = [Ttok[c][:, gt] for c in range(2)]

                # ---- qkv projections (heads pair-stacked on partitions) ----
                from contextlib import nullcontext
                _hp = tc.high_priority(offset=_prio) if _prio > 0 else nullcontext()
                QA = ps.tile([128, NT], F32, name="QA", tag="ps")
                QB = ps.tile([128, NT], F32, name="QB", tag="ps")
                KA = ps.tile([128, NT], F32, name="KA", tag="ps")
                KB = ps.tile([128, NT], F32, name="KB", tag="ps")
                with _hp:
                    for dst, Wsrc, h in ((QA, Wq, 0), (KA, Wk, 0), (QB, Wq, 1), (KB, Wk, 1)):
                        for c in range(2):
                            nc.tensor.matmul(dst[:], Wsrc[c][h][:], tok[c], start=(c == 0), stop=(c == 1))
                qA = grp.tile([128, NT], BF16, name="qA", tag="qA")
                qB = grp.tile([128, NT], BF16, name="qB", tag="qB")
                kA = grp.tile([128, NT], BF16, name="kA", tag="kA")
                kB = grp.tile([128, NT], BF16, name="kB", tag="kB")
                nc.scalar.copy(qA[:], QA[:])
                nc.scalar.copy(kA[:], KA[:])
                nc.scalar.copy(qB[:], QB[:])
                nc.scalar.copy(kB[:], KB[:])

                if not stage_ge("v"):
                    nc.vector.tensor_copy(OS[0][:, gt], QA[:])
                    nc.vector.tensor_copy(OS[1][:, gt], KA[:])
                    continue
                # ---- v (token-rows form), one psum tile per window-pair ----
                VP = [ps.tile([128, 256], F32, name=f"VP{p}", tag="ps") for p in range(4)]
                for p in range(4):
                    for c in range(2):
                        lhsT = Ttok[c][:, g * NT + p * 128: g * NT + (p + 1) * 128]
                        nc.tensor.matmul(VP[p][:], lhsT, Wv[c][:], start=(c == 0), stop=(c == 1))
                vP = [grp.tile([128, 512], BF16, name=f"vP{i}", tag=f"vP{i}") for i in range(2)]
                for p in range(4):
                    nc.vector.tensor_copy(vP[p // 2][:, (p % 2) * 256:(p % 2 + 1) * 256], VP[p][:])

                if not stage_ge("sim"):
                    nc.vector.tensor_copy(OS[0][:, gt], vP[0][:])
                    nc.vector.tensor_copy(OS[1][:, gt], vP[1][:])
                    continue
                # ---- sim (per window, heads pair-stacked via quadrants) ----
                SA = ps.tile([128, NT], F32, name="SA", tag="ps")
                SB = ps.tile([128, NT], F32, name="SB", tag="ps")
                for w in range(GW):
                    wt = slice(w * T, (w + 1) * T)
                    for hh in range(2):
                        pp = slice(hh * 64, (hh + 1) * 64)
                        nc.tensor.matmul(SA[pp, wt], qA[pp, wt], kA[pp, wt], start=True, stop=True)
                        nc.tensor.matmul(SB[pp, wt], qB[pp, wt], kB[pp, wt], start=True, stop=True)

                if not stage_ge("soft"):
                    nc.vector.tensor_copy(OS[0][:, gt], SA[:])
                    nc.vector.tensor_copy(OS[1][:, gt], SB[:])
                    continue
                # ---- softmax over t_k (free axis), rows layout ----
                for S, bias, aTname in ((SA, biasA, "A"), (SB, biasB, "B")):
                    Eu = soft.tile([128, NT], BF16, name=f"Eu{aTname}", tag=f"Eu{aTname}")
                    E16 = soft.tile([128, NT], BF16, name=f"E16{aTname}", tag=f"E16{aTname}")
                    rs = soft.tile([128, GW], F32, name=f"rs{aTname}", tag=f"rs{aTname}")
                    rr = soft.tile([128, GW], F32, name=f"rr{aTname}", tag=f"rr{aTname}")
                    N16 = soft.tile([128, NT], BF16, name=f"N16{aTname}", tag=f"N16{aTname}")
                    nc.scalar.activation(Eu[:], S[:], EXP)
                    nc.vector.tensor_mul(E16[:], Eu[:], bias[:])
                    nc.vector.reduce_sum(rs[:], E16[:].rearrange("p (w k) -> p w k", w=GW), axis=AX)
                    nc.vector.reciprocal(rr[:], rs[:])
                    _norm_eng.tensor_mul(
                        N16[:].rearrange("p (w k) -> p w k", w=GW),
                        E16[:].rearrange("p (w k) -> p w k", w=GW),
                        rr[:].unsqueeze(2).broadcast_to([128, GW, T]),
                    )
                    if aTname == "A":
                        NA16 = N16
                    else:
                        NB16 = N16

                if not stage_ge("tp"):
                    nc.vector.tensor_copy(OS[0][:, gt], NA16[:])
                    nc.vector.tensor_copy(OS[1][:, gt], NB16[:])
                    continue
                # ---- transpose attn per window-pair -> (wl*64+t_k, hh*64+t_q) ----
                TA = ps.tile([128, NT], BF16, name="TA", tag="ps")
                TB = ps.tile([128, NT], BF16, name="TB", tag="ps")
                for p in range(4):
                    isl = slice(p * 128, (p + 1) * 128)
                    nc.tensor.transpose(TA[:, isl], NA16[:, isl], ident[:])
                    nc.tensor.transpose(TB[:, isl], NB16[:, isl], ident[:])
                aT = grp.tile([128, NT], BF16, name="aTA", tag="aTA")
                bT = grp.tile([128, NT], BF16, name="aTB", tag="aTB")
                nc.vector.tensor_copy(aT[:], TA[:])
                nc.vector.tensor_copy(bT[:], TB[:])

                if not stage_ge("av"):
                    nc.vector.tensor_copy(OS[0][:, gt], aT[:])
                    nc.vector.tensor_copy(OS[1][:, gt], bT[:])
                    continue
                # ---- attn @ v -> o rows (t_q on partitions, pair-stacked);
                # every matmul strictly diagonal: lhsT/rhs/out share base wl*64
                OA = ps.tile([128, NT], F32, name="OA", tag="ps")
                OB = ps.tile([128, NT], F32, name="OB", tag="ps")
                for p in range(4):
                    for wl in range(2):
                        ksl = slice(wl * 64, (wl + 1) * 64)
                        for hh in range(2):
                            csl = slice(p * 128 + hh * 64, p * 128 + hh * 64 + 64)
                            nc.tensor.matmul(
                                OA[ksl, csl],
                                aT[ksl, p * 128 + hh * 64: p * 128 + (hh + 1) * 64],
                                vP[p // 2][ksl, (p % 2) * 256 + hh * 64:(p % 2) * 256 + (hh + 1) * 64],
                                start=True, stop=True)
                            nc.tensor.matmul(
                                OB[ksl, csl],
                                bT[ksl, p * 128 + hh * 64: p * 128 + (hh + 1) * 64],
                                vP[p // 2][ksl, (p % 2) * 256 + 128 + hh * 64:(p % 2) * 256 + 128 + (hh + 1) * 64],
                                start=True, stop=True)
                oRa = grp.tile([128, NT], BF16, name="oRa", tag="oRa")
                oRb = grp.tile([128, NT], BF16, name="oRb", tag="oRb")
                nc.vector.tensor_copy(oRa[:], OA[:])
                nc.vector.tensor_copy(oRb[:], OB[:])

                # ---- transpose o rows -> oT (cin on partitions, token cols) ----
                TPA = ps.tile([128, NT], BF16, name="TPA", tag="ps")
                TPB = ps.tile([128, NT], BF16, name="TPB", tag="ps")
                for p in range(4):
                    isl = slice(p * 128, (p + 1) * 128)
                    nc.tensor.transpose(TPA[:, isl], oRa[:, isl], ident[:])
                    nc.tensor.transpose(TPB[:, isl], oRb[:, isl], ident[:])
                oA = grp.tile([128, NT], BF16, name="oA", tag="oA")
                oB = grp.tile([128, NT], BF16, name="oB", tag="oB")
                nc.scalar.copy(oA[:], TPA[:])
                nc.scalar.copy(oB[:], TPB[:])

                if not stage_ge("out"):
                    nc.vector.tensor_copy(OS[0][:, gt], oA[:])
                    nc.vector.tensor_copy(OS[1][:, gt], oB[:])
                    continue
                # ---- out projection: outT (cout, tokens) ----
                UA = ps.tile([128, NT], F32, name="UA", tag="ps")
                UB = ps.tile([128, NT], F32, name="UB", tag="ps")
                for c, o_ in ((0, oA), (1, oB)):
                    st, sp = (c == 0), (c == 1)
                    nc.tensor.matmul(UA[:], Wo[c][0][:], o_[:], start=st, stop=sp)
                    nc.tensor.matmul(UB[:], Wo[c][1][:], o_[:], start=st, stop=sp)
                nc.scalar.copy(OS[0][:, gt], UA[:])
                nc.scalar.copy(OS[1][:, gt], UB[:])

            # reorder back (ww, a, i, j) -> (a, i, w128) and DMA out
            chunk_out = True
            for c in range(2):
                orv = OR[c][:].rearrange("p (a i w j) -> p a w i j", a=4, i=4, w=32, j=4)
                osv = OS[c][:].rearrange("p (w a i j) -> p a w i j", w=32, a=4, i=4, j=4)
                ord_ = OR[c][:].rearrange("p (a i w) -> p a i w", a=4, i=4)
                if chunk_out:
                    # drain the final strip per group so the tail is not gated
                    # on the whole strip finishing
                    for gch in range(N_GROUPS_):
                        wsl8 = slice(gch * 8, (gch + 1) * 8)
                        wsl32 = slice(gch * 32, (gch + 1) * 32)
                        for a_ in range(4):
                            nc.gpsimd.tensor_copy(orv[:, a_, wsl8], osv[:, a_, wsl8])
                            nc.sync.dma_start(
                                out=out_d[a_, c * 128:(c + 1) * 128, hs_sl, wsl32],
                                in_=ord_[:, a_, :, wsl32])
                else:
                    for a_ in range(4):
                        nc.gpsimd.tensor_copy(orv[:, a_], osv[:, a_])
                    dst = out_d[:, c * 128:(c + 1) * 128, hs_sl, :].transpose([1, 0, 2, 3])
                    nc.sync.dma_start(out=dst, in_=ord_)

    return _split_multi_waits(nc)


_NC_CACHE = None


def kernel(x, w_qkv, w_out, bias_table, _want_trace=False):
    global _NC_CACHE
    from concourse.bass_utils import run_bass_kernel_spmd

    x = np.asarray(x, dtype=np.float32)
    w_qkv = np.asarray(w_qkv, dtype=np.float32)
    w_out = np.asarray(w_out, dtype=np.float32)
    bias_table = np.asarray(bias_table, dtype=np.float32)

    scale = (DIM // HEADS) ** -0.5
    wq = np.ascontiguousarray(w_qkv[:, 0:DIM] * scale)
    wk = np.ascontiguousarray(w_qkv[:, DIM:2 * DIM])
    wv = np.ascontiguousarray(w_qkv[:, 2 * DIM:3 * DIM])
    biasA, biasB = _build_bias_stacks(bias_table)
    ident = np.eye(128, dtype=np.float32)

    if _NC_CACHE is None:
        _NC_CACHE = build_nc()
    nc = _NC_CACHE

    in_maps = []
    for m in range(N_CORES):
        xs = np.ascontiguousarray(x[:, :, m * HS:(m + 1) * HS, :])
        in_maps.append({
            "xs": xs, "wq": wq, "wk": wk, "wv": wv, "wo": np.ascontiguousarray(w_out),
            "biasA": biasA, "biasB": biasB, "ident": ident,
        })
    res = run_bass_kernel_spmd(nc, in_maps, list(range(N_CORES)), trace=_want_trace)
    out = np.empty((N_AGENTS, DIM, H, W), dtype=np.float32)
    for m in range(N_CORES):
        out[:, :, m * HS:(m + 1) * HS, :] = res.results[m]["out"]
    if _want_trace:
        return out, res
    return out



# revision 17
# speedup vs baseline: 1.6948x; 1.6948x over previous
"""Trainium2 Bass kernel for windowed multi-agent attention (Swin-style).

Full-input contract: kernel(**inputs) takes the unsharded inputs and returns
the unsharded output. Internally shards over the H axis across 8 NeuronCores
(fully data-parallel over window rows), builds one SPMD Bass program, and
runs it via run_bass_kernel_spmd.

Host-side, x is pre-reordered into per-core token-major layout (bf16):
  xs[core][c2, p128, strip4, (w32 a4 i4 j4)]
so each (c, strip) loads with a single fully-contiguous DMA and the SBUF
tile is already in window-token order. The output uses the same layout in
reverse (bf16), reassembled to NCHW f32 on the host.

Per window (4x4 spatial, 4 agents => T=64 tokens; the padded 5th agent is
masked out everywhere in the reference, so it is dropped):
  xw (64, 256) -> qkv -> 4 heads of d=64 -> softmax(q k^T * scale + bias) v
  -> out proj (256, 256).

The attention core is computed fully in "transposed" (S^T) space to avoid
PE transposes:
  ST = k^T q                      (t_k on partitions, t_q on cols)
  E  = exp(ST) * exp(B)^T         (bias multiplicative, host-precomputed)
  Z  = sel^T @ E                  (PE matmul with 64x64 block mask ->
                                   per-(window, t_q) sums replicated across
                                   the 64 t_k partitions of each half)
  P^T = E * reciprocal(Z)
  o^T = v^T P^T                   (lhsT = v with t_k on partitions)
  out^T = Wo^T o^T
"""

import numpy as np

HEADS = 4
WIN = 4
MAX_N = 5
DIM = 256
N_AGENTS = 4
H = W = 128
N_CORES = 8
T = N_AGENTS * WIN * WIN          # 64 valid tokens per window
HS = 16                           # H rows per core
N_STRIPS = 4                      # window-rows per core (4 H-rows each)
N_GROUPS = 4                      # groups of 8 windows per strip
GW = 8                            # windows per group
NT = GW * T                       # tokens per group = 512
SW = 32                           # windows per strip
STOK = SW * T                     # tokens per strip = 2048


def _rel_pos_index(N, wh, ww, md, mh, mw):
    cd, ch, cw = np.arange(N), np.arange(wh), np.arange(ww)
    coords = np.stack(np.meshgrid(cd, ch, cw, indexing="ij")).reshape(3, -1)
    rel = (coords[:, :, None] - coords[:, None, :]).transpose(1, 2, 0).astype(np.int64)
    rel[..., 0] += md - 1
    rel[..., 1] += mh - 1
    rel[..., 2] += mw - 1
    rel[..., 0] *= (2 * mh - 1) * (2 * mw - 1)
    rel[..., 1] *= 2 * mw - 1
    return rel.sum(-1)


def _build_biasT_stacks(bias_table):
    """Transposed multiplicative bias stacks, one per head-pair 'stack'.

    Returns (2, 128, NT) f32 where
      out[st, hh*64 + tk, w*64 + tq] = exp(B_{2*st+hh}[tq, tk])
    (independent of w: identical 64x64 blocks tiled along the 8 windows)."""
    rpi = _rel_pos_index(MAX_N, WIN, WIN, MAX_N, WIN, WIN)   # (80, 80)
    b = bias_table[rpi]                                      # (80, 80, HEADS)
    b = b[:T, :T].transpose(2, 0, 1).astype(np.float32)      # (HEADS, tq, tk)
    out = np.zeros((2, 128, NT), np.float32)
    for st in range(2):
        for hh in range(2):
            blk = np.exp(b[2 * st + hh].T)                   # (tk, tq)
            for w in range(GW):
                out[st, hh * 64:(hh + 1) * 64, w * T:(w + 1) * T] = blk
    return np.ascontiguousarray(out)


def _patch_tile_drain():
    """Walrus in this container rejects >1 sync-wait on the TileContext tail
    drain; split the waits across individual SP nops instead."""
    from concourse import tile as tile_mod
    from concourse.vector_clock import ScopedClock, VectorClock
    if getattr(tile_mod.TileContext, "_drain_patched", False):
        return

    def _patched(self, tick_clock, wait_clock):
        gc_ = tick_clock.global_clock
        n = len(gc_)
        for proc in range(n):
            tick = gc_[proc]
            if tick <= 0:
                continue
            vc = VectorClock([0] * n)
            vc.require_at_least(proc, tick)
            nop_inst = self.nc.sync.nop(nofuse=True)
            wait_clock.add_sem_waits(nop_inst.ins, ScopedClock({None: vc}))
        self.nc.sync.drain()
        self.nc.all_engine_barrier()
        popped = self.nc._tile_sem_poison_stack.pop()
        assert popped is self._sem_poison
        self.nc.clear_and_free_semaphores(list(self.sems.allocated().values()))
        self.nc.all_engine_barrier()

    tile_mod.TileContext._drain_and_barrier = _patched
    tile_mod.TileContext._drain_patched = True


def _split_multi_waits(nc):
    """Walrus here allows only one sync-wait per instruction. Rewrite the BIR
    json: for each instruction with >1 on_wait, hoist the extras onto fresh
    single-wait Nops inserted just before it on the same engine."""
    import orjson
    orig = nc.to_json_bytes

    def patched():
        bj = orjson.loads(orig())
        counter = [0]
        for fn in bj.get("functions", []):
            for blk in fn.get("blocks", []):
                insts = blk.get("instructions", [])
                out = []
                for inst in insts:
                    si = inst.get("sync_info") or {}
                    waits = si.get("on_wait") or []
                    if len(waits) > 1:
                        for w in waits[:-1]:
                            counter[0] += 1
                            out.append({
                                "name": f"WSPL-{counter[0]}",
                                "opcode": "NoOp",
                                "engine": inst["engine"],
                                "ins": [],
                                "outs": [],
                                "sync_info": {"on_update": [], "on_wait": [w]},
                            })
                        si["on_wait"] = [waits[-1]]
                    out.append(inst)
                blk["instructions"] = out
        return orjson.dumps(bj)

    nc.to_json_bytes = patched
    return nc


def build_nc():
    import os
    from concourse import bass, mybir
    from concourse.tile import TileContext
    _patch_tile_drain()
    KMULS = os.environ.get("KMULS", "dve")      # dve | pool | split
    KCHUNK0 = os.environ.get("KCHUNK0", "0") == "1"
    KDIV = os.environ.get("KDIV", "0") == "1"   # tensor_tensor divide for norm
    KOS = os.environ.get("KOS", "act")          # act | split : U->OS evac engines

    F32 = mybir.dt.float32
    BF16 = mybir.dt.bfloat16
    EXP = mybir.ActivationFunctionType.Exp

    nc = bass.Bass("TRN2", target_bir_lowering=False, debug=False,
                   num_devices=N_CORES)

    xs_d = nc.dram_tensor("xs", [2, 128, N_STRIPS, STOK], BF16, kind="ExternalInput").ap()
    wq_d = nc.dram_tensor("wq", [DIM, DIM], F32, kind="ExternalInput").ap()
    wk_d = nc.dram_tensor("wk", [DIM, DIM], F32, kind="ExternalInput").ap()
    wv_d = nc.dram_tensor("wv", [DIM, DIM], F32, kind="ExternalInput").ap()
    wo_d = nc.dram_tensor("wo", [DIM, DIM], F32, kind="ExternalInput").ap()
    bt_d = nc.dram_tensor("biasT", [2, 128, NT], F32, kind="ExternalInput").ap()
    sel_d = nc.dram_tensor("sel", [128, 128], F32, kind="ExternalInput").ap()
    out_d = nc.dram_tensor("out", [2, 128, N_STRIPS, STOK], BF16, kind="ExternalOutput").ap()

    from contextlib import ExitStack
    with TileContext(nc) as tc, ExitStack() as _stk:
        cpool = _stk.enter_context(tc.tile_pool(name="consts", bufs=1))
        Wq = [[cpool.tile([128, 128], BF16, name=f"wq{c}{h}", tag=f"wq{c}{h}") for h in range(2)] for c in range(2)]
        Wk = [[cpool.tile([128, 128], BF16, name=f"wk{c}{h}", tag=f"wk{c}{h}") for h in range(2)] for c in range(2)]
        Wv = [cpool.tile([128, 256], BF16, name=f"wv{c}", tag=f"wv{c}") for c in range(2)]
        Wo = [[cpool.tile([128, 128], BF16, name=f"wo{c}{h}", tag=f"wo{c}{h}") for h in range(2)] for c in range(2)]
        biasT = [cpool.tile([128, NT], BF16, name=f"biasT{s}", tag=f"biasT{s}") for s in range(2)]
        sel = cpool.tile([128, 128], BF16, name="sel", tag="sel")
        for c in range(2):
            cs = slice(c * 128, (c + 1) * 128)
            for h in range(2):
                hs_ = slice(h * 128, (h + 1) * 128)
                nc.gpsimd.dma_start(out=Wq[c][h][:], in_=wq_d[cs, hs_])
                nc.gpsimd.dma_start(out=Wk[c][h][:], in_=wk_d[cs, hs_])
                nc.gpsimd.dma_start(out=Wo[c][h][:], in_=wo_d[cs, hs_])
            nc.gpsimd.dma_start(out=Wv[c][:], in_=wv_d[cs, :])
        for s in range(2):
            nc.gpsimd.dma_start(out=biasT[s][:], in_=bt_d[s])
        nc.gpsimd.dma_start(out=sel[:], in_=sel_d)

        strip = _stk.enter_context(tc.tile_pool(name="strip", bufs=2))
        grp = _stk.enter_context(tc.tile_pool(name="grp", bufs=3))
        # Two 4-bank PSUM pools; even/odd groups alternate so the FIFO tag
        # rotation pairs same-stage tiles two periods apart (no early-stage
        # of group g+1 waiting on a late-stage bank of group g).
        psp = [_stk.enter_context(tc.tile_pool(name=f"ps{i}", bufs=4, space="PSUM"))
               for i in range(2)]
        gidx = [0]

        def load_strip(s, chunked=False):
            Traw = [strip.tile([128, STOK], BF16, name=f"traw{c}", tag=f"traw{c}") for c in range(2)]
            if chunked:
                # per-group chunks so group 0 compute starts early
                for g in range(N_GROUPS):
                    gsl = slice(g * NT, (g + 1) * NT)
                    for c in range(2):
                        nc.sync.dma_start(out=Traw[c][:, gsl], in_=xs_d[c, :, s, gsl])
            else:
                for c in range(2):
                    nc.sync.dma_start(out=Traw[c][:], in_=xs_d[c, :, s, :])
            return Traw

        pending = load_strip(0, chunked=KCHUNK0)
        for s in range(N_STRIPS):
            Traw = pending
            if s + 1 < N_STRIPS:
                pending = load_strip(s + 1)
            OS = [strip.tile([128, STOK], BF16, name=f"os{c}", tag=f"os{c}") for c in range(2)]

            for g in range(N_GROUPS):
                gt = slice(g * NT, (g + 1) * NT)
                tok = [Traw[c][:, gt] for c in range(2)]
                ps = psp[gidx[0] % 2]
                gidx[0] += 1

                # ---- q/k projections (head-pair stacks on partitions) ----
                # PSUM tile alloc order must follow stage order (see pool note)
                QA = ps.tile([128, NT], F32, name="QA", tag="ps")
                QB = ps.tile([128, NT], F32, name="QB", tag="ps")
                KA = ps.tile([128, NT], F32, name="KA", tag="ps")
                KB = ps.tile([128, NT], F32, name="KB", tag="ps")
                VP = [ps.tile([128, 512], F32, name=f"VP{i}", tag="ps") for i in range(2)]
                for dst, Wsrc, h in ((QA, Wq, 0), (KA, Wk, 0), (QB, Wq, 1), (KB, Wk, 1)):
                    for c in range(2):
                        nc.tensor.matmul(dst[:], Wsrc[c][h][:], tok[c], start=(c == 0), stop=(c == 1))
                # ---- v (token-rows form), windows pair-stacked on partitions ----
                for p in range(4):
                    dst = VP[p // 2][:, (p % 2) * 256:(p % 2 + 1) * 256]
                    for c in range(2):
                        lhsT = Traw[c][:, g * NT + p * 128: g * NT + (p + 1) * 128]
                        nc.tensor.matmul(dst, lhsT, Wv[c][:], start=(c == 0), stop=(c == 1))

                qA = grp.tile([128, NT], BF16, name="qA", tag="qA")
                qB = grp.tile([128, NT], BF16, name="qB", tag="qB")
                kA = grp.tile([128, NT], BF16, name="kA", tag="kA")
                kB = grp.tile([128, NT], BF16, name="kB", tag="kB")
                nc.scalar.copy(qA[:], QA[:])
                nc.scalar.copy(kA[:], KA[:])
                nc.scalar.copy(qB[:], QB[:])
                nc.scalar.copy(kB[:], KB[:])
                # v in SBUF: window-pair tokens on partitions x 2x256 chans;
                # vSW = half-swapped copy (window tokens at the opposite
                # partition half) via SBUF->SBUF DMA so every AV matmul can be
                # partition-diagonal (off-diagonal PE tile placement faults).
                vSB1 = grp.tile([128, 1024], BF16, name="vSB1", tag="vSB1")
                nc.vector.tensor_copy(vSB1[:, 0:512], VP[0][:])
                nc.vector.tensor_copy(vSB1[:, 512:1024], VP[1][:])
                vSW = grp.tile([128, 1024], BF16, name="vSW", tag="vSW")
                nc.sync.dma_start(out=vSW[0:64, :], in_=vSB1[64:128, :])
                nc.sync.dma_start(out=vSW[64:128, :], in_=vSB1[0:64, :])

                # ---- sim: ST = k^T q, (hh, t_k) partitions x (w, t_q) cols
                # (partition-diagonal: out/lhsT/rhs all at base hh*64)
                SA = ps.tile([128, NT], F32, name="SA", tag="ps")
                SB = ps.tile([128, NT], F32, name="SB", tag="ps")
                for w in range(GW):
                    wt = slice(w * T, (w + 1) * T)
                    for hh in range(2):
                        hsl = slice(hh * 64, (hh + 1) * 64)
                        nc.tensor.matmul(SA[hsl, wt], kA[hsl, wt], qA[hsl, wt], start=True, stop=True)
                        nc.tensor.matmul(SB[hsl, wt], kB[hsl, wt], qB[hsl, wt], start=True, stop=True)

                # ---- softmax pieces in transposed space ----
                EuA = grp.tile([128, NT], BF16, name="EuA", tag="EuA")
                EuB = grp.tile([128, NT], BF16, name="EuB", tag="EuB")
                nc.scalar.activation(EuA[:], SA[:], EXP)
                nc.scalar.activation(EuB[:], SB[:], EXP)
                EBA = grp.tile([128, NT], BF16, name="EBA", tag="EBA")
                EBB = grp.tile([128, NT], BF16, name="EBB", tag="EBB")
                _bias_eng = nc.gpsimd if KMULS in ("pool", "split") else nc.vector
                _bias_eng.tensor_mul(EBA[:], EuA[:], biasT[0][:])
                _bias_eng.tensor_mul(EBB[:], EuB[:], biasT[1][:])

                ZA = ps.tile([128, NT], F32, name="ZA", tag="ps")
                ZB = ps.tile([128, NT], F32, name="ZB", tag="ps")
                nc.tensor.matmul(ZA[:], sel[:], EBA[:], start=True, stop=True)
                nc.tensor.matmul(ZB[:], sel[:], EBB[:], start=True, stop=True)
                rzA = grp.tile([128, NT], BF16, name="rzA", tag="rzA")
                rzB = grp.tile([128, NT], BF16, name="rzB", tag="rzB")
                with nc.allow_low_precision(reason="softmax denom, bf16 ok at 2e-2 tol"):
                    nc.vector.reciprocal(rzA[:], ZA[:])
                    nc.vector.reciprocal(rzB[:], ZB[:])
                NTA = grp.tile([128, NT], BF16, name="NTA", tag="NTA")
                NTB = grp.tile([128, NT], BF16, name="NTB", tag="NTB")
                _norm_eng = nc.gpsimd if KMULS == "pool" else nc.vector
                _norm_eng.tensor_mul(NTA[:], EBA[:], rzA[:])
                _norm_eng.tensor_mul(NTB[:], EBB[:], rzB[:])

                # ---- o^T = v^T P^T: (hh, dv) partitions x (w, t_q) cols ----
                # diagonal at base hh*64; pick vSB1 or the half-swapped vSW so
                # window w's tokens sit at partition half hh.
                OTA = ps.tile([128, NT], F32, name="OTA", tag="ps")
                OTB = ps.tile([128, NT], F32, name="OTB", tag="ps")
                for w in range(GW):
                    wt = slice(w * T, (w + 1) * T)
                    p = w // 2
                    cbase = (p // 2) * 512 + (p % 2) * 256
                    for hh in range(2):
                        hsl = slice(hh * 64, (hh + 1) * 64)
                        vt = vSB1 if (w % 2) == hh else vSW
                        nc.tensor.matmul(
                            OTA[hsl, wt],
                            vt[hsl, cbase + hh * 64: cbase + (hh + 1) * 64],
                            NTA[hsl, wt], start=True, stop=True)
                        nc.tensor.matmul(
                            OTB[hsl, wt],
                            vt[hsl, cbase + 128 + hh * 64: cbase + 128 + (hh + 1) * 64],
                            NTB[hsl, wt], start=True, stop=True)
                oA = grp.tile([128, NT], BF16, name="oA", tag="oA")
                oB = grp.tile([128, NT], BF16, name="oB", tag="oB")
                nc.vector.tensor_copy(oA[:], OTA[:])
                nc.vector.tensor_copy(oB[:], OTB[:])

                # ---- out projection: out^T (cout, tokens) ----
                UA = ps.tile([128, NT], F32, name="UA", tag="ps")
                UB = ps.tile([128, NT], F32, name="UB", tag="ps")
                for c, o_ in ((0, oA), (1, oB)):
                    st_, sp_ = (c == 0), (c == 1)
                    nc.tensor.matmul(UA[:], Wo[c][0][:], o_[:], start=st_, stop=sp_)
                    nc.tensor.matmul(UB[:], Wo[c][1][:], o_[:], start=st_, stop=sp_)
                nc.scalar.copy(OS[0][:, gt], UA[:])
                nc.scalar.copy(OS[1][:, gt], UB[:])

            for c in range(2):
                nc.sync.dma_start(out=out_d[c, :, s, :], in_=OS[c][:])

    return _split_multi_waits(nc)


_NC_CACHE = None


def kernel(x, w_qkv, w_out, bias_table, _want_trace=False):
    global _NC_CACHE
    import ml_dtypes
    from concourse.bass_utils import run_bass_kernel_spmd

    BF = ml_dtypes.bfloat16
    x = np.asarray(x, dtype=np.float32)
    w_qkv = np.asarray(w_qkv, dtype=np.float32)
    w_out = np.asarray(w_out, dtype=np.float32)
    bias_table = np.asarray(bias_table, dtype=np.float32)

    scale = (DIM // HEADS) ** -0.5
    wq = np.ascontiguousarray(w_qkv[:, 0:DIM] * scale)
    wk = np.ascontiguousarray(w_qkv[:, DIM:2 * DIM])
    wv = np.ascontiguousarray(w_qkv[:, 2 * DIM:3 * DIM])
    biasT = _build_biasT_stacks(bias_table)
    selm = np.zeros((128, 128), np.float32)
    selm[:64, :64] = 1.0
    selm[64:, 64:] = 1.0

    # host reorder: x (a, C, H, W) -> per-core (c2, p128, s4, (w a i j)) bf16
    xr = x.reshape(N_AGENTS, 2, 128, N_CORES, N_STRIPS, WIN, SW, WIN)
    xr = xr.transpose(3, 1, 2, 4, 6, 0, 5, 7)     # (m, c, p, s, w, a, i, j)
    xr = np.ascontiguousarray(xr.reshape(N_CORES, 2, 128, N_STRIPS, STOK)).astype(BF)

    if _NC_CACHE is None:
        _NC_CACHE = build_nc()
    nc = _NC_CACHE

    in_maps = []
    for m in range(N_CORES):
        in_maps.append({
            "xs": xr[m], "wq": wq, "wk": wk, "wv": wv,
            "wo": np.ascontiguousarray(w_out),
            "biasT": biasT, "sel": selm,
        })
    res = run_bass_kernel_spmd(nc, in_maps, list(range(N_CORES)), trace=_want_trace)
    out = np.empty((N_AGENTS, DIM, H, W), dtype=np.float32)
    for m in range(N_CORES):
        o = np.asarray(res.results[m]["out"], dtype=np.float32)
        o = o.reshape(2, 128, N_STRIPS, SW, N_AGENTS, WIN, WIN)   # c p s w a i j
        o = o.transpose(4, 0, 1, 2, 5, 3, 6)                      # a c p s i w j
        out[:, :, m * HS:(m + 1) * HS, :] = o.reshape(N_AGENTS, DIM, HS, W)
    if _want_trace:
        return out, res
    return out


# revision 32
# speedup vs baseline: 2.0127x; 1.1876x over previous
"""Trainium2 Bass kernel for windowed multi-agent attention (Swin-style).

Full-input contract: kernel(**inputs) takes the unsharded inputs and returns
the unsharded output. Internally shards over the H axis across 8 NeuronCores
(fully data-parallel over window rows), builds one SPMD Bass program, and
runs it via run_bass_kernel_spmd.

Host-side, x is pre-reordered into per-core token-major layout (bf16):
  xs[core][c2, p128, strip4, (w32 a4 i4 j4)]
so each (c, strip) loads with a single fully-contiguous DMA and the SBUF
tile is already in window-token order. The output uses the same layout in
reverse (bf16), reassembled to NCHW f32 on the host.

Per window (4x4 spatial, 4 agents => T=64 tokens; the padded 5th agent is
masked out everywhere in the reference, so it is dropped):
  xw (64, 256) -> qkv -> 4 heads of d=64 -> softmax(q k^T * scale + bias) v
  -> out proj (256, 256).

The attention core is computed fully in "transposed" (S^T) space to avoid
PE transposes:
  ST = k^T q                      (t_k on partitions, t_q on cols)
  E  = exp(ST) * exp(B)^T         (bias multiplicative, host-precomputed)
  Z  = sel^T @ E                  (PE matmul with 64x64 block mask ->
                                   per-(window, t_q) sums replicated across
                                   the 64 t_k partitions of each half)
  P^T = E * reciprocal(Z)
  o^T = v^T P^T                   (lhsT = v with t_k on partitions)
  out^T = Wo^T o^T
"""

import numpy as np

HEADS = 4
WIN = 4
MAX_N = 5
DIM = 256
N_AGENTS = 4
H = W = 128
N_CORES = 8
T = N_AGENTS * WIN * WIN          # 64 valid tokens per window
HS = 16                           # H rows per core
N_STRIPS = 4                      # window-rows per core (4 H-rows each)
N_GROUPS = 4                      # groups of 8 windows per strip
GW = 8                            # windows per group
NT = GW * T                       # tokens per group = 512
SW = 32                           # windows per strip
STOK = SW * T                     # tokens per strip = 2048


def _rel_pos_index(N, wh, ww, md, mh, mw):
    cd, ch, cw = np.arange(N), np.arange(wh), np.arange(ww)
    coords = np.stack(np.meshgrid(cd, ch, cw, indexing="ij")).reshape(3, -1)
    rel = (coords[:, :, None] - coords[:, None, :]).transpose(1, 2, 0).astype(np.int64)
    rel[..., 0] += md - 1
    rel[..., 1] += mh - 1
    rel[..., 2] += mw - 1
    rel[..., 0] *= (2 * mh - 1) * (2 * mw - 1)
    rel[..., 1] *= 2 * mw - 1
    return rel.sum(-1)


def _build_biasT_stacks(bias_table):
    """Transposed multiplicative bias stacks, one per head-pair 'stack'.

    Returns (2, 128, NT) f32 where
      out[st, hh*64 + tk, w*64 + tq] = exp(B_{2*st+hh}[tq, tk])
    (independent of w: identical 64x64 blocks tiled along the 8 windows)."""
    rpi = _rel_pos_index(MAX_N, WIN, WIN, MAX_N, WIN, WIN)   # (80, 80)
    b = bias_table[rpi]                                      # (80, 80, HEADS)
    b = b[:T, :T].transpose(2, 0, 1).astype(np.float32)      # (HEADS, tq, tk)
    out = np.zeros((2, 128, NT), np.float32)
    for st in range(2):
        for hh in range(2):
            blk = np.exp(b[2 * st + hh].T)                   # (tk, tq)
            for w in range(GW):
                out[st, hh * 64:(hh + 1) * 64, w * T:(w + 1) * T] = blk
    return np.ascontiguousarray(out)


def _patch_tile_drain():
    """Walrus in this container rejects >1 sync-wait on the TileContext tail
    drain; split the waits across individual SP nops instead."""
    from concourse import tile as tile_mod
    from concourse.vector_clock import ScopedClock, VectorClock
    if getattr(tile_mod.TileContext, "_drain_patched", False):
        return

    def _patched(self, tick_clock, wait_clock):
        gc_ = tick_clock.global_clock
        n = len(gc_)
        for proc in range(n):
            tick = gc_[proc]
            if tick <= 0:
                continue
            vc = VectorClock([0] * n)
            vc.require_at_least(proc, tick)
            nop_inst = self.nc.sync.nop(nofuse=True)
            wait_clock.add_sem_waits(nop_inst.ins, ScopedClock({None: vc}))
        self.nc.sync.drain()
        self.nc.all_engine_barrier()
        popped = self.nc._tile_sem_poison_stack.pop()
        assert popped is self._sem_poison
        self.nc.clear_and_free_semaphores(list(self.sems.allocated().values()))
        self.nc.all_engine_barrier()

    tile_mod.TileContext._drain_and_barrier = _patched
    tile_mod.TileContext._drain_patched = True


def _split_multi_waits(nc):
    """Walrus here allows only one sync-wait per instruction. Rewrite the BIR
    json: for each instruction with >1 on_wait, hoist the extras onto fresh
    single-wait Nops inserted just before it on the same engine."""
    import orjson
    orig = nc.to_json_bytes

    def patched():
        bj = orjson.loads(orig())
        counter = [0]
        for fn in bj.get("functions", []):
            for blk in fn.get("blocks", []):
                insts = blk.get("instructions", [])
                out = []
                for inst in insts:
                    si = inst.get("sync_info") or {}
                    waits = si.get("on_wait") or []
                    if len(waits) > 1:
                        for w in waits[:-1]:
                            counter[0] += 1
                            out.append({
                                "name": f"WSPL-{counter[0]}",
                                "opcode": "NoOp",
                                "engine": inst["engine"],
                                "ins": [],
                                "outs": [],
                                "sync_info": {"on_update": [], "on_wait": [w]},
                            })
                        si["on_wait"] = [waits[-1]]
                    out.append(inst)
                blk["instructions"] = out
        return orjson.dumps(bj)

    nc.to_json_bytes = patched
    return nc


def build_nc():
    import os
    from concourse import bass, mybir
    from concourse.tile import TileContext
    _patch_tile_drain()
    KMULS = os.environ.get("KMULS", "dve")      # dve | pool | split
    KCHUNK0 = os.environ.get("KCHUNK0", "1") == "1"
    KDIV = os.environ.get("KDIV", "0") == "1"   # tensor_tensor divide for norm
    KOS = os.environ.get("KOS", "split")        # act | split : U->OS evac engines
    KVSB = os.environ.get("KVSB", "split")      # dve | act | split : VP evac
    KOUTCHUNK = os.environ.get("KOUTCHUNK", "1") == "1"
    KFP8 = os.environ.get("KFP8", "0") == "1"   # DoubleRow fp8 V-proj + out-proj

    F32 = mybir.dt.float32
    BF16 = mybir.dt.bfloat16
    EXP = mybir.ActivationFunctionType.Exp

    nc = bass.Bass("TRN2", target_bir_lowering=False, debug=False,
                   num_devices=N_CORES)

    F8 = mybir.dt.float8e4
    xs_d = nc.dram_tensor("xs", [2, 128, N_STRIPS, STOK], BF16, kind="ExternalInput").ap()
    # one packed bf16 constant blob: [wq(4x128) wk(4x128) wv(2x256) wo(4x128)
    #  biasT(2x512) sel(128)] = 3200 cols (single DMA; 17 small HWDGE-
    # serialized DMAs made the pipeline fill weights-bound)
    wc_d = nc.dram_tensor("wcat", [128, 3200], BF16, kind="ExternalInput").ap()
    if KFP8:
        # fp8 copies for DoubleRow: x tokens + [wv8 (2x256) | wo8 (2x256)]
        xs8_d = nc.dram_tensor("xs8", [2, 128, N_STRIPS, STOK], F8, kind="ExternalInput").ap()
        w8_d = nc.dram_tensor("w8", [128, 1024], F8, kind="ExternalInput").ap()
    out_d = nc.dram_tensor("out", [2, 128, N_STRIPS, STOK], BF16, kind="ExternalOutput").ap()

    from contextlib import ExitStack
    with TileContext(nc) as tc, ExitStack() as _stk:
        cpool = _stk.enter_context(tc.tile_pool(name="consts", bufs=1))
        wcat = cpool.tile([128, 3200], BF16, name="wcat", tag="wcat")
        nc.sync.dma_start(out=wcat[:], in_=wc_d)

        def _wslice(base, n):
            return wcat[:, base:base + n]
        Wq = [[_wslice((c * 2 + h) * 128, 128) for h in range(2)] for c in range(2)]
        Wk = [[_wslice(512 + (c * 2 + h) * 128, 128) for h in range(2)] for c in range(2)]
        Wv = [_wslice(1024 + c * 256, 256) for c in range(2)]
        Wo = [[_wslice(1536 + (c * 2 + h) * 128, 128) for h in range(2)] for c in range(2)]
        biasT = [_wslice(2048 + s * NT, NT) for s in range(2)]
        sel = _wslice(3072, 128)
        if KFP8:
            w8 = cpool.tile([128, 1024], F8, name="w8", tag="w8")
            nc.sync.dma_start(out=w8[:], in_=w8_d)
            # 3D [p, 2(c-half), N] views for DoubleRow
            Wv8 = w8[:, 0:512].rearrange("p (two n) -> p two n", two=2)
            Wo8 = [w8[:, 512 + h * 256: 512 + (h + 1) * 256].rearrange(
                "p (two n) -> p two n", two=2) for h in range(2)]

        strip = _stk.enter_context(tc.tile_pool(name="strip", bufs=2))
        grp = _stk.enter_context(tc.tile_pool(name="grp", bufs=3))
        # Two 4-bank PSUM pools; even/odd groups alternate so the FIFO tag
        # rotation pairs same-stage tiles two periods apart (no early-stage
        # of group g+1 waiting on a late-stage bank of group g).
        psp = [_stk.enter_context(tc.tile_pool(name=f"ps{i}", bufs=4, space="PSUM"))
               for i in range(2)]
        gidx = [0]

        def load_strip(s, chunked=False):
            Traw = [strip.tile([128, STOK], BF16, name=f"traw{c}", tag=f"traw{c}") for c in range(2)]
            if chunked:
                # per-group chunks so group 0 compute starts early
                for g in range(N_GROUPS):
                    gsl = slice(g * NT, (g + 1) * NT)
                    for c in range(2):
                        nc.sync.dma_start(out=Traw[c][:, gsl], in_=xs_d[c, :, s, gsl])
            else:
                for c in range(2):
                    nc.sync.dma_start(out=Traw[c][:], in_=xs_d[c, :, s, :])
            return Traw

        pending = load_strip(0, chunked=KCHUNK0)
        for s in range(N_STRIPS):
            Traw = pending
            if s + 1 < N_STRIPS:
                pending = load_strip(s + 1)
            OS = [strip.tile([128, STOK], BF16, name=f"os{c}", tag=f"os{c}") for c in range(2)]

            for g in range(N_GROUPS):
                gt = slice(g * NT, (g + 1) * NT)
                tok = [Traw[c][:, gt] for c in range(2)]
                ps = psp[gidx[0] % 2]
                gidx[0] += 1

                # ---- q/k projections (head-pair stacks on partitions) ----
                # PSUM tile alloc order must follow stage order (see pool note)
                QA = ps.tile([128, NT], F32, name="QA", tag="ps")
                QB = ps.tile([128, NT], F32, name="QB", tag="ps")
                KA = ps.tile([128, NT], F32, name="KA", tag="ps")
                KB = ps.tile([128, NT], F32, name="KB", tag="ps")
                VP = [ps.tile([128, 512], F32, name=f"VP{i}", tag="ps") for i in range(2)]
                for dst, Wsrc, h in ((QA, Wq, 0), (KA, Wk, 0), (QB, Wq, 1), (KB, Wk, 1)):
                    for c in range(2):
                        nc.tensor.matmul(dst[:], Wsrc[c][h], tok[c], start=(c == 0), stop=(c == 1))
                # ---- v (token-rows form), windows pair-stacked on partitions ----
                for p in range(4):
                    dst = VP[p // 2][:, (p % 2) * 256:(p % 2 + 1) * 256]
                    for c in range(2):
                        lhsT = Traw[c][:, g * NT + p * 128: g * NT + (p + 1) * 128]
                        nc.tensor.matmul(dst, lhsT, Wv[c], start=(c == 0), stop=(c == 1))

                qA = grp.tile([128, NT], BF16, name="qA", tag="qA")
                qB = grp.tile([128, NT], BF16, name="qB", tag="qB")
                kA = grp.tile([128, NT], BF16, name="kA", tag="kA")
                kB = grp.tile([128, NT], BF16, name="kB", tag="kB")
                nc.scalar.copy(qA[:], QA[:])
                nc.scalar.copy(kA[:], KA[:])
                nc.scalar.copy(qB[:], QB[:])
                nc.scalar.copy(kB[:], KB[:])
                # v in SBUF: window-pair tokens on partitions x 2x256 chans;
                # vSW = half-swapped copy (window tokens at the opposite
                # partition half) via SBUF->SBUF DMA so every AV matmul can be
                # partition-diagonal (off-diagonal PE tile placement faults).
                vSB1 = grp.tile([128, 1024], BF16, name="vSB1", tag="vSB1")
                if KVSB == "split":
                    nc.scalar.copy(vSB1[:, 0:512], VP[0][:])
                    nc.vector.tensor_copy(vSB1[:, 512:1024], VP[1][:])
                elif KVSB == "act":
                    nc.scalar.copy(vSB1[:, 0:512], VP[0][:])
                    nc.scalar.copy(vSB1[:, 512:1024], VP[1][:])
                else:
                    nc.vector.tensor_copy(vSB1[:, 0:512], VP[0][:])
                    nc.vector.tensor_copy(vSB1[:, 512:1024], VP[1][:])
                vSW = grp.tile([128, 1024], BF16, name="vSW", tag="vSW")
                nc.sync.dma_start(out=vSW[0:64, :], in_=vSB1[64:128, :])
                nc.sync.dma_start(out=vSW[64:128, :], in_=vSB1[0:64, :])

                # ---- sim: ST = k^T q, (hh, t_k) partitions x (w, t_q) cols
                # (partition-diagonal: out/lhsT/rhs all at base hh*64)
                SA = ps.tile([128, NT], F32, name="SA", tag="ps")
                SB = ps.tile([128, NT], F32, name="SB", tag="ps")
                for w in range(GW):
                    wt = slice(w * T, (w + 1) * T)
                    for hh in range(2):
                        hsl = slice(hh * 64, (hh + 1) * 64)
                        nc.tensor.matmul(SA[hsl, wt], kA[hsl, wt], qA[hsl, wt], start=True, stop=True)
                        nc.tensor.matmul(SB[hsl, wt], kB[hsl, wt], qB[hsl, wt], start=True, stop=True)

                # ---- softmax pieces in transposed space ----
                EuA = grp.tile([128, NT], BF16, name="EuA", tag="EuA")
                EuB = grp.tile([128, NT], BF16, name="EuB", tag="EuB")
                nc.scalar.activation(EuA[:], SA[:], EXP)
                nc.scalar.activation(EuB[:], SB[:], EXP)
                EBA = grp.tile([128, NT], BF16, name="EBA", tag="EBA")
                EBB = grp.tile([128, NT], BF16, name="EBB", tag="EBB")
                _bias_eng = nc.gpsimd if KMULS in ("pool", "split") else nc.vector
                _bias_eng.tensor_mul(EBA[:], EuA[:], biasT[0])
                _bias_eng.tensor_mul(EBB[:], EuB[:], biasT[1])

                ZA = ps.tile([128, NT], F32, name="ZA", tag="ps")
                ZB = ps.tile([128, NT], F32, name="ZB", tag="ps")
                nc.tensor.matmul(ZA[:], sel, EBA[:], start=True, stop=True)
                nc.tensor.matmul(ZB[:], sel, EBB[:], start=True, stop=True)
                NTA = grp.tile([128, NT], BF16, name="NTA", tag="NTA")
                NTB = grp.tile([128, NT], BF16, name="NTB", tag="NTB")
                if KDIV:
                    from concourse import mybir as _mb
                    nc.vector.tensor_tensor(NTA[:], EBA[:], ZA[:], op=_mb.AluOpType.divide)
                    nc.vector.tensor_tensor(NTB[:], EBB[:], ZB[:], op=_mb.AluOpType.divide)
                else:
                    rzA = grp.tile([128, NT], BF16, name="rzA", tag="rzA")
                    rzB = grp.tile([128, NT], BF16, name="rzB", tag="rzB")
                    with nc.allow_low_precision(reason="softmax denom, bf16 ok at 2e-2 tol"):
                        nc.vector.reciprocal(rzA[:], ZA[:])
                        nc.vector.reciprocal(rzB[:], ZB[:])
                    _norm_eng = nc.gpsimd if KMULS == "pool" else nc.vector
                    _norm_eng.tensor_mul(NTA[:], EBA[:], rzA[:])
                    _norm_eng.tensor_mul(NTB[:], EBB[:], rzB[:])

                # ---- o^T = v^T P^T: (hh, dv) partitions x (w, t_q) cols ----
                # diagonal at base hh*64; pick vSB1 or the half-swapped vSW so
                # window w's tokens sit at partition half hh.
                OTA = ps.tile([128, NT], F32, name="OTA", tag="ps")
                OTB = ps.tile([128, NT], F32, name="OTB", tag="ps")
                for w in range(GW):
                    wt = slice(w * T, (w + 1) * T)
                    p = w // 2
                    cbase = (p // 2) * 512 + (p % 2) * 256
                    for hh in range(2):
                        hsl = slice(hh * 64, (hh + 1) * 64)
                        vt = vSB1 if (w % 2) == hh else vSW
                        nc.tensor.matmul(
                            OTA[hsl, wt],
                            vt[hsl, cbase + hh * 64: cbase + (hh + 1) * 64],
                            NTA[hsl, wt], start=True, stop=True)
                        nc.tensor.matmul(
                            OTB[hsl, wt],
                            vt[hsl, cbase + 128 + hh * 64: cbase + 128 + (hh + 1) * 64],
                            NTB[hsl, wt], start=True, stop=True)
                oA = grp.tile([128, NT], BF16, name="oA", tag="oA")
                oB = grp.tile([128, NT], BF16, name="oB", tag="oB")
                nc.vector.tensor_copy(oA[:], OTA[:])
                nc.vector.tensor_copy(oB[:], OTB[:])

                # ---- out projection: out^T (cout, tokens) ----
                UA = ps.tile([128, NT], F32, name="UA", tag="ps")
                UB = ps.tile([128, NT], F32, name="UB", tag="ps")
                for c, o_ in ((0, oA), (1, oB)):
                    st_, sp_ = (c == 0), (c == 1)
                    nc.tensor.matmul(UA[:], Wo[c][0], o_[:], start=st_, stop=sp_)
                    nc.tensor.matmul(UB[:], Wo[c][1], o_[:], start=st_, stop=sp_)
                nc.scalar.copy(OS[0][:, gt], UA[:])
                if KOS == "split":
                    nc.vector.tensor_copy(OS[1][:, gt], UB[:])
                else:
                    nc.scalar.copy(OS[1][:, gt], UB[:])
                if KOUTCHUNK:
                    for c in range(2):
                        nc.sync.dma_start(out=out_d[c, :, s, gt], in_=OS[c][:, gt])

            if not KOUTCHUNK:
                for c in range(2):
                    nc.sync.dma_start(out=out_d[c, :, s, :], in_=OS[c][:])

    return _split_multi_waits(nc)


_NC_CACHE = None


def kernel(x, w_qkv, w_out, bias_table, _want_trace=False):
    global _NC_CACHE
    import ml_dtypes
    from concourse.bass_utils import run_bass_kernel_spmd

    BF = ml_dtypes.bfloat16
    x = np.asarray(x, dtype=np.float32)
    w_qkv = np.asarray(w_qkv, dtype=np.float32)
    w_out = np.asarray(w_out, dtype=np.float32)
    bias_table = np.asarray(bias_table, dtype=np.float32)

    scale = (DIM // HEADS) ** -0.5
    wq = w_qkv[:, 0:DIM] * scale
    wk = w_qkv[:, DIM:2 * DIM]
    wv = w_qkv[:, 2 * DIM:3 * DIM]
    biasT = _build_biasT_stacks(bias_table)
    selm = np.zeros((128, 128), np.float32)
    selm[:64, :64] = 1.0
    selm[64:, 64:] = 1.0

    # packed constant blob (128, 3200): [wq wk wv wo biasT sel], each 256-row
    # weight split into two 128-partition column blocks
    def _split_c(wmat):       # (256, n) -> (128, 2*n)
        return np.concatenate([wmat[0:128, :], wmat[128:256, :]], axis=1)
    wcat = np.concatenate([
        _split_c(wq), _split_c(wk), _split_c(wv), _split_c(w_out),
        biasT[0], biasT[1], selm,
    ], axis=1).astype(BF)
    assert wcat.shape == (128, 3200)

    # host reorder: x (a, C, H, W) -> per-core (c2, p128, s4, (w a i j)) bf16
    xr = x.reshape(N_AGENTS, 2, 128, N_CORES, N_STRIPS, WIN, SW, WIN)
    xr = xr.transpose(3, 1, 2, 4, 6, 0, 5, 7)     # (m, c, p, s, w, a, i, j)
    xr = np.ascontiguousarray(xr.reshape(N_CORES, 2, 128, N_STRIPS, STOK)).astype(BF)

    if _NC_CACHE is None:
        _NC_CACHE = build_nc()
    nc = _NC_CACHE

    in_maps = []
    for m in range(N_CORES):
        in_maps.append({"xs": xr[m], "wcat": wcat})
    res = run_bass_kernel_spmd(nc, in_maps, list(range(N_CORES)), trace=_want_trace)
    out = np.empty((N_AGENTS, DIM, H, W), dtype=np.float32)
    for m in range(N_CORES):
        o = np.asarray(res.results[m]["out"], dtype=np.float32)
        o = o.reshape(2, 128, N_STRIPS, SW, N_AGENTS, WIN, WIN)   # c p s w a i j
        o = o.transpose(4, 0, 1, 2, 5, 3, 6)                      # a c p s i w j
        out[:, :, m * HS:(m + 1) * HS, :] = o.reshape(N_AGENTS, DIM, HS, W)
    if _want_trace:
        return out, res
    return out
